# revision 29
# baseline (speedup 1.0000x reference)
"""Causal self-attention with RoPE on 8 Trainium2 NeuronCores.

Sharding: tensor-parallel over heads (2 heads/core) through QKV projection,
RoPE and attention; AllToAll reshards attention output from head-split to
token-split; out-projection is token-parallel with full out_w per core
(no reduction needed). Output: each core produces its 512-token slice.

Schedule: QKV-projection blocks and attention chunks are fused into one
software-pipelined stream (see the comment at the fused loop); the softmax
denominator is accumulated on DVE (bf16 4x) and finished on Pool
(partition_all_reduce) so the PE only runs projections, scores, PV and the
out-projection.

Layouts (per core, f = feature, t = token, d = contraction):
  xT   [D, NT]    input transposed (d on partitions) - rhs/lhsT for projections
  q/k  [128, NT]  per-head, head-dim on partitions ("qT"): proj out [f, t]
  v    [NT, 256]  token-major: proj out [t, f]
  sT   [j, r]     scores transposed: lhsT=kT-tile, rhs=qT-block
  pT   [j, r]     exp(scores*scale) bf16, causally trimmed to r >= 128*(jt-4rb)
  oT   [dv, r]    PV: lhsT=v-tile [j, dv], rhs=pT [j, r]
  dacc [j, r]     bf16 elementwise-accumulated pT (denominator before reduce)
  out  [t, e]     out-proj: lhsT=attnT-tile [dv, t], rhs=out_wT [dv, e]
"""
import math
import numpy as np
import ml_dtypes

import concourse.bass as bass
import concourse.mybir as mybir
import concourse.tile as tile
from concourse import bacc, bass_isa
from concourse.bass_utils import run_bass_kernel_spmd

F32 = mybir.dt.float32
F32R = mybir.dt.float32r
BF16 = mybir.dt.bfloat16
AF = mybir.ActivationFunctionType
ALU = mybir.AluOpType

N_CORES = 8


def legalize_waits(nc, max_waits=1):
    """This walrus build only encodes one sync-wait per TPB instruction.
    Move extra waits emitted by Tile onto same-engine NoOps inserted
    immediately before the instruction."""
    n_split = 0
    for fn in nc.m.functions:
        for bb in fn.blocks:
            new_insts = []
            for inst in bb.instructions:
                si = getattr(inst, "sync_info", None)
                waits = list(si.on_wait) if si is not None and si.on_wait else []
                if len(waits) > max_waits and type(inst).__name__ != "InstNoOp":
                    extra, keep = waits[:-max_waits], waits[-max_waits:]
                    for k, w in enumerate(extra):
                        nop = mybir.InstNoOp(
                            name=f"{inst.name}_waitnop{k}",
                            engine=inst.engine,
                            ins=[],
                            outs=[],
                            sync_info=mybir.SyncInfo(on_wait=[w], on_update=[]),
                        )
                        nc.register_instruction(nop)
                        new_insts.append(nop)
                    inst.sync_info = mybir.SyncInfo(
                        on_wait=keep, on_update=list(si.on_update)
                    )
                    n_split += 1
                new_insts.append(inst)
            bb.instructions = new_insts
    return n_split


def build_nc(B=2, T=2048, D=2048, H=16, fake_cc=False, n_loop=1, dummy_io=False):
    HD = D // H                  # 128, head dim
    NT = B * T                   # total tokens
    HPC = H // N_CORES           # heads per core (2)
    DC = HPC * HD                # head channels per core (256)
    KT = D // 128                # contraction tiles for projections (16)
    NB = NT // 512               # 512-token blocks overall (8)
    RB = T // 512                # 512-token blocks per batch element (4)
    S = NT // N_CORES            # AllToAll shard = tokens per core (512)
    EB = D // 512                # 512-wide out-feature blocks (4)
    SCALE = 1.0 / math.sqrt(HD)

    nc = bacc.Bacc("TRN2", target_bir_lowering=False, debug=False, num_devices=N_CORES)
    # dummy_io: declare data tensors as internal DRAM (uninitialized) so the
    # timing NEFF has no big inputs to ship through the axon tunnel.
    ik = {"kind": "ExternalInput"} if not dummy_io else {}
    xT_e = nc.dram_tensor("xT", [D, NT], BF16, **ik)
    wqk_e = nc.dram_tensor("wqk", [D, 4 * HD], BF16, **ik)
    bqk_e = nc.dram_tensor("bqk", [4 * HD], F32, **ik)
    wv_e = nc.dram_tensor("wv", [D, DC], BF16, **ik)
    bv_e = nc.dram_tensor("bv", [DC], F32, **ik)
    cos_e = nc.dram_tensor("cosT", [HD, NT], BF16, **ik)
    sin_e = nc.dram_tensor("sinT", [HD, NT], BF16, **ik)
    masks_e = nc.dram_tensor("masks", [4, 128, 512], BF16, **ik)
    owT_e = nc.dram_tensor("owT", [D, D], BF16, **ik)
    ob_e = nc.dram_tensor("ob", [D], F32, **ik)
    out_e = nc.dram_tensor("out", [S, D], F32, kind="ExternalOutput")

    with tile.TileContext(nc) as tc:
      for _it in range(n_loop):
        with tc.tile_pool(name=f"persist{_it}", bufs=1) as pp, \
             tc.tile_pool(name=f"ow0{_it}", bufs=1) as owp0, \
             tc.tile_pool(name=f"dram{_it}", bufs=1, space="DRAM") as dp:
            # ---- persistent tiles ----
            qk = [pp.tile([128, NT], BF16, tag=f"qk{m}", name=f"qk{m}") for m in range(4)]
            v_sb = pp.tile([128, NT // 128, DC], BF16, tag="v", name="v")
            masks = pp.tile([128, 4, 512], BF16, tag="masks", name="masks")
            bqk = pp.tile([128, 4], F32, tag="bqk", name="bqk")
            nc.sync.dma_start(bqk[:], bqk_e.rearrange("(m p) -> p m", p=128))
            bv1 = pp.tile([1, DC], F32, tag="bv1", name="bv1")
            nc.sync.dma_start(bv1[:], bv_e[None, :])
            bvB = pp.tile([128, DC], F32, tag="bvB", name="bvB")
            nc.gpsimd.partition_broadcast(bvB[:], bv1[:])
            ob1 = pp.tile([1, D], F32, tag="ob1", name="ob1")
            obB = pp.tile([128, D], F32, tag="obB", name="obB")

            Zs = [dp.tile([N_CORES, HD, S], BF16, tag=f"Z{i}", name=f"Z{i}")
                  for i in range(HPC)]
            ZGs = [dp.tile([N_CORES, HD, S], BF16, tag=f"ZG{i}", name=f"ZG{i}")
                   for i in range(HPC)]

            zg_sb = [pp.tile([128, N_CORES, S], BF16, tag=f"zg{i}", name=f"zg{i}")
                     for i in range(HPC)]

            # ---- fused phases 1+2: QKV projection blocks interleaved with
            # attention chunks ----
            # Phase 1 alone is ~100% PE-bound while ACT/DVE idle; phase 2
            # alone is ACT/DVE-chain-bound with the PE only ~55% busy. After
            # projection block i (tokens [512i, 512i+512)), the attention
            # row-block (b=i//4, rb=i%4) has all its keys/values, so its
            # chunks are interleaved between block i+1's projection units:
            # the exp/mask/denominator chain hides under projection matmuls.
            # Attention work is emitted per single key-tile (128 keys x 512
            # queries) with a K_SKEW-deep software pipeline so the in-order
            # PE never waits on the ACT exp. Causal trim: key-tile jt only
            # serves queries q >= 128*(jt-4rb). The softmax denominator is
            # accumulated on DVE (bf16 4x mode) and finished on Pool
            # (partition_all_reduce + reciprocal) - no PE involvement.
            K_SKEW = 3
            with tc.tile_pool(name=f"p1w{_it}", bufs=1) as wp, \
                 tc.tile_pool(name=f"p1x{_it}", bufs=2) as xp, \
                 tc.tile_pool(name=f"p1t{_it}", bufs=3) as tp, \
                 tc.tile_pool(name=f"p2t{_it}", bufs=4) as tp2, \
                 tc.tile_pool(name=f"p2r{_it}", bufs=2) as tpr, \
                 tc.tile_pool(name=f"p1ps{_it}", bufs=2, space="PSUM") as ps1, \
                 tc.tile_pool(name=f"p2ps{_it}", bufs=4, space="PSUM") as ps2, \
                 tc.tile_pool(name=f"p2po{_it}", bufs=2, space="PSUM") as ps2o:
                wqk_sb = wp.tile([128, KT, 4 * HD], BF16, tag="wqk", name="wqk")
                wqk_r = wqk_e.rearrange("(kt p) f -> p kt f", p=128)
                for m2 in range(2):
                    for kh in range(2):
                        nc.scalar.dma_start(
                            wqk_sb[:, bass.ts(kh, KT // 2), bass.ts(m2, 256)],
                            wqk_r[:, bass.ts(kh, KT // 2), bass.ts(m2, 256)])
                # cos/sin halves: batch-0 tokens are needed ~25us in
                # (RoPE of block 0 gates the first attention chunks), batch-1
                # tokens only after ~80us. wv by the first v unit (~14us).
                cos_sb = wp.tile([128, NT], BF16, tag="cos", name="cos")
                sin_sb = wp.tile([128, NT], BF16, tag="sin", name="sin")
                wv_sb = wp.tile([128, KT, DC], BF16, tag="wv", name="wv")
                nc.scalar.dma_start(cos_sb[:, :T], cos_e[:, :T])
                nc.scalar.dma_start(sin_sb[:, :T], sin_e[:, :T])
                nc.scalar.dma_start(wv_sb[:], wv_e.rearrange("(kt p) f -> p kt f", p=128))
                nc.scalar.dma_start(cos_sb[:, T:], cos_e[:, T:])
                nc.scalar.dma_start(sin_sb[:, T:], sin_e[:, T:])

                state = {}

                def emit_p1_unit(blk, u, xb):
                    tsl = bass.ts(blk, 512)
                    psu = ps1.tile([128, 512], F32, tag="ps", name="ps")
                    if u < 4:
                        m = u
                        for kt in range(KT):
                            nc.tensor.matmul(
                                psu[:],
                                wqk_sb[:, kt, bass.ts(m, 128)],
                                xb[:, kt, :],
                                start=(kt == 0),
                                stop=(kt == KT - 1),
                            )
                        nc.scalar.activation(
                            qk[m][:, tsl], psu[:], AF.Identity,
                            bias=bqk[:, m:m + 1], scale=1.0,
                        )
                        # RoPE in place: qk = qk*cos + swap(qk)*s2, where
                        # s2 = sin with first half negated (host-prepared) and
                        # swap exchanges partition halves (engines cannot read
                        # across partitions -> use SBUF->SBUF DMA).
                        qm = qk[m][:, tsl]
                        qsw = tp.tile([128, 512], BF16, tag="qsw", name="qsw")
                        nc.sync.dma_start(qsw[0:64, :], qm[64:128, :])
                        nc.sync.dma_start(qsw[64:128, :], qm[0:64, :])
                        nc.vector.tensor_mul(qsw[:], qsw[:], sin_sb[:, tsl])
                        nc.vector.tensor_mul(qm, qm, cos_sb[:, tsl])
                        nc.vector.tensor_add(qm, qm, qsw[:])
                    else:
                        tt = u - 4
                        for kt in range(KT):
                            nc.tensor.matmul(
                                psu[:, :DC],
                                xb[:, kt, bass.ts(tt, 128)],
                                wv_sb[:, kt, :],
                                start=(kt == 0),
                                stop=(kt == KT - 1),
                            )
                        nc.vector.tensor_add(v_sb[:, blk * 4 + tt, :], psu[:, :DC], bvB[:])

                def a_chunks(blk):
                    b, rb = blk // 4, blk % 4
                    return [(hh, b, rb, jt) for hh in range(HPC)
                            for jt in range(4 * rb + 4)]

                def emit_ascores(c):
                    hh, b, rb, jt = c
                    q0 = max(jt - 4 * rb, 0) * 128
                    pss = ps2.tile([128, 512], F32, tag="pss", name="pss")
                    nc.tensor.matmul(
                        pss[:, bass.ds(q0, 512 - q0)],
                        qk[2 + hh][:, bass.ds(b * T + jt * 128, 128)],
                        qk[hh][:, bass.ds(b * T + rb * 512 + q0, 512 - q0)],
                        start=True, stop=True,
                    )
                    state[c] = pss

                def emit_apost(c):
                    hh, b, rb, jt = c
                    pss = state.pop(c)
                    njt = 4 * rb + 4
                    if jt == 0:
                        state[c[:3]] = (
                            ps2o.tile([128, 512], F32, tag="pso", name="pso"),
                            tpr.tile([128, 512], BF16, tag="dacc", name="dacc"),
                        )
                    pso, dacc = state[c[:3]]
                    mh = jt - 4 * rb
                    q0 = max(mh, 0) * 128
                    csl = bass.ds(q0, 512 - q0)
                    pT = tp2.tile([128, 512], BF16, tag="pT", name="pT")
                    nc.scalar.activation(pT[:, csl], pss[:, csl], AF.Exp, scale=SCALE)
                    if mh >= 0:
                        nc.vector.tensor_mul(pT[:, csl], pT[:, csl], masks[:, mh, csl])
                    if jt == 0:
                        nc.vector.tensor_copy(dacc[:], pT[:])
                    else:
                        nc.vector.tensor_add(dacc[:, csl], dacc[:, csl], pT[:, csl])
                    nc.tensor.matmul(
                        pso[:, csl],
                        v_sb[:, (b * T) // 128 + jt, bass.ts(hh, HD)],
                        pT[:, csl],
                        start=(jt == 0),
                        stop=(jt == njt - 1),
                    )
                    if jt != njt - 1:
                        return
                    del state[c[:3]]
                    # denominator finish on Pool (off the PE path): all-reduce
                    # over partitions -> reciprocal -> scale
                    denB = tpr.tile([128, 512], F32, tag="denB", name="denB")
                    nc.gpsimd.partition_all_reduce(
                        denB[:], dacc[:], 128, bass_isa.ReduceOp.add)
                    recipB = tpr.tile([128, 512], F32, tag="recipB", name="recipB")
                    nc.vector.reciprocal(recipB[:], denB[:])
                    oT = tpr.tile([128, 512], BF16, tag="oT", name="oT")
                    nc.vector.tensor_mul(oT[:], pso[:], recipB[:])
                    g = b * RB + rb
                    for off in range(0, 512, S):
                        sh = (g * 512 + off) // S
                        nc.sync.dma_start(
                            Zs[hh][sh, :, bass.ds((g * 512 + off) % S, min(512, S))],
                            oT[:, bass.ds(off, min(512, S))],
                        )
                    if b == B - 1 and rb == RB - 1:
                        # reshard this head-half; phase 4 consumes it
                        if fake_cc:
                            nc.scalar.dma_start(ZGs[hh][:], Zs[hh][:])
                        else:
                            nc.gpsimd.collective_compute(
                                "AllToAll", ALU.bypass,
                                replica_groups=[list(range(N_CORES))],
                                ins=[Zs[hh][:]], outs=[ZGs[hh][:]],
                            )
                        nc.sync.dma_start(
                            zg_sb[hh][:], ZGs[hh][:].rearrange("c d s -> d c s")
                        )

                pend = []

                def push_chunk(c):
                    emit_ascores(c)
                    pend.append(c)
                    if len(pend) > K_SKEW:
                        emit_apost(pend.pop(0))

                last = a_chunks(NB - 1)
                # hh0 chunks of the last block whose KEY tile lives in an
                # earlier block interleave with the last block's v-projection
                # units; ALL hh1 chunks stay after hh0's AllToAll so the
                # collective flies under real attention work
                last_early = [c for c in last if c[3] < 4 * (RB - 1) and c[0] == 0]
                last_late = [c for c in last if c[3] >= 4 * (RB - 1) or c[0] != 0]
                n_le, le_done = len(last_early), 0
                for blk in range(NB):
                    xb = xp.tile([128, KT, 512], BF16, tag="xb", name="xb")
                    xTr = xT_e.rearrange("(kt p) t -> p kt t", p=128)
                    tsl = bass.ts(blk, 512)
                    npc = 4 if blk == 0 else 2
                    for pc in range(npc):
                        ksl = bass.ts(pc, KT // npc)
                        nc.sync.dma_start(xb[:, ksl, :], xTr[:, ksl, tsl])
                    if blk == 0:
                        nc.sync.dma_start(masks[:], masks_e.rearrange("m p c -> p m c"))
                    # block-0 attention is deferred to block 2: at block
                    # 1 the startup weight/mask DMAs are still in flight
                    if blk < 2:
                        ach = []
                    elif blk == 2:
                        ach = a_chunks(0) + a_chunks(1)
                    else:
                        ach = a_chunks(blk - 1)
                    done = 0
                    for u in range(8):
                        emit_p1_unit(blk, u, xb)
                        # at the last block, drain the previous block's chunks
                        # by unit 3 so its own early chunks (keys in earlier
                        # blocks) can follow without interleaving groups
                        den = 4 if blk == NB - 1 else 8
                        want = min(len(ach), (len(ach) * (u + 1) + den - 1) // den)
                        while done < want:
                            push_chunk(ach[done])
                            done += 1
                        if blk == NB - 1 and u >= 4:
                            want_e = (n_le * (u - 3) + 3) // 4
                            while le_done < want_e:
                                push_chunk(last_early[le_done])
                                le_done += 1
                # out-proj bias setup here: any earlier and the obB
                # broadcast head-of-line-blocks the denominator all_reduces
                # on the in-order Pool queue
                nc.scalar.dma_start(ob1[:], ob_e[None, :])
                nc.gpsimd.partition_broadcast(obB[:], ob1[:])
                # prefetch the first out-proj weight tile (pass 1, e=0)
                # while the attention tail drains
                ow0 = owp0.tile([128, KT // 2, 512], BF16, tag="ow0", name="ow0")
                owr0 = owT_e.rearrange("(kt p) f -> p kt f", p=128)[:, :, bass.ts(0, 512)]
                nc.scalar.dma_start(ow0[:], owr0[:, 0::2, :])
                for c in last_early[le_done:] + last_late:
                    push_chunk(c)
                while pend:
                    emit_apost(pend.pop(0))

            # ---- phase 4: out projection on own token slice ----
            # Two-pass contraction: all zg0 (first AllToAll) partial sums are
            # computed and evicted before any zg1 tile is touched, so the
            # second AllToAll and the zg1 load hide behind real matmul work.
            with tc.tile_pool(name=f"p4z{_it}", bufs=1) as zp, \
                 tc.tile_pool(name=f"p4w{_it}", bufs=3) as owp, \
                 tc.tile_pool(name=f"p4t{_it}", bufs=4) as tp4, \
                 tc.tile_pool(name=f"p4ps{_it}", bufs=4, space="PSUM") as ps4:
                accbig = zp.tile([128, EB * (S // 128), 512], F32, tag="accbig", name="accbig")
                zgs = zg_sb
                for e in range(EB):
                    if e == 0:
                        ow = ow0
                    else:
                        ow = owp.tile([128, KT // 2, 512], BF16, tag="ow", name="ow")
                        owr = owT_e.rearrange("(kt p) f -> p kt f", p=128)[:, :, bass.ts(e, 512)]
                        ow_even = owr[:, 0::2, :]
                        nc.scalar.dma_start(ow[:, :KT // 4, :], ow_even[:, :KT // 4, :])
                        nc.scalar.dma_start(ow[:, KT // 4:, :], ow_even[:, KT // 4:, :])
                    for tt in range(S // 128):
                        pso4 = ps4.tile([128, 512], F32, tag="ps4", name="ps4")
                        for zt in range(KT // HPC):
                            nc.tensor.matmul(
                                pso4[:],
                                zgs[0][:, zt, bass.ts(tt, 128)],
                                ow[:, zt, :],
                                start=(zt == 0),
                                stop=(zt == KT // HPC - 1),
                            )
                        nc.scalar.activation(
                            accbig[:, e * (S // 128) + tt, :], pso4[:], AF.Copy)
                for e in range(EB):
                    ow2 = owp.tile([128, KT // 2, 512], BF16, tag="ow2", name="ow2")
                    owr = owT_e.rearrange("(kt p) f -> p kt f", p=128)[:, :, bass.ts(e, 512)]
                    ow_odd = owr[:, 1::2, :]
                    nc.scalar.dma_start(ow2[:, :KT // 4, :], ow_odd[:, :KT // 4, :])
                    nc.scalar.dma_start(ow2[:, KT // 4:, :], ow_odd[:, KT // 4:, :])
                    for tt in range(S // 128):
                        pso4 = ps4.tile([128, 512], F32, tag="ps4", name="ps4")
                        for zt in range(KT // HPC):
                            nc.tensor.matmul(
                                pso4[:],
                                zgs[1][:, zt, bass.ts(tt, 128)],
                                ow2[:, zt, :],
                                start=(zt == 0),
                                stop=(zt == KT // HPC - 1),
                            )
                        mid = tp4.tile([128, 512], F32, tag="mid", name="mid")
                        nc.vector.tensor_add(mid[:], pso4[:], accbig[:, e * (S // 128) + tt, :])
                        of = tp4.tile([128, 512], F32, tag="of", name="of")
                        nc.vector.tensor_add(of[:], mid[:], obB[:, bass.ts(e, 512)])
                        nc.sync.dma_start(out_e[bass.ts(tt, 128), bass.ts(e, 512)], of[:])

    nc.compile()          # Bacc pass pipeline (library loads, nop fusion, regs)
    legalize_waits(nc)    # must run after all nop-fusion passes
    bass.Bass.finalize(nc)  # freeze without re-running Bacc compile
    return nc


def _prep_inputs(x, rope_cos, rope_sin, qkv_w, qkv_b, out_w, out_b, B, T, D, H):
    HD = D // H
    NT = B * T
    HPC = H // N_CORES
    bf = ml_dtypes.bfloat16

    x2 = np.ascontiguousarray(x.reshape(NT, D).T).astype(bf)           # [D, NT]
    cosT = np.ascontiguousarray(
        np.tile(rope_cos[0, 0].T, (1, B))).astype(bf)                   # [HD, NT]
    s2 = np.tile(rope_sin[0, 0].T, (1, B)).copy()
    s2[:HD // 2] *= -1.0
    sinT = np.ascontiguousarray(s2).astype(bf)
    owT = np.ascontiguousarray(out_w.T).astype(bf)                      # [D, D]
    ob = out_b.astype(np.float32)

    c_grid = np.arange(512)[None, :]
    p_grid = np.arange(128)[:, None]
    masks = np.stack(
        [(c_grid >= 128 * m + p_grid) for m in range(4)]
    ).astype(bf)                                                        # [4,128,512]

    in_maps = []
    for c in range(N_CORES):
        heads = [HPC * c + i for i in range(HPC)]
        q_rows = np.concatenate([qkv_w[h * HD:(h + 1) * HD] for h in heads])
        k_rows = np.concatenate([qkv_w[D + h * HD:D + (h + 1) * HD] for h in heads])
        v_rows = np.concatenate([qkv_w[2 * D + h * HD:2 * D + (h + 1) * HD] for h in heads])
        wqk = np.ascontiguousarray(np.concatenate([q_rows, k_rows]).T).astype(bf)
        wv = np.ascontiguousarray(v_rows.T).astype(bf)
        bq = np.concatenate([qkv_b[h * HD:(h + 1) * HD] for h in heads])
        bk = np.concatenate([qkv_b[D + h * HD:D + (h + 1) * HD] for h in heads])
        bqk = np.concatenate([bq, bk]).astype(np.float32)
        bv = np.concatenate(
            [qkv_b[2 * D + h * HD:2 * D + (h + 1) * HD] for h in heads]
        ).astype(np.float32)
        in_maps.append({
            "xT": x2, "wqk": wqk, "bqk": bqk, "wv": wv, "bv": bv,
            "cosT": cosT, "sinT": sinT, "masks": masks,
            "owT": owT, "ob": ob,
        })
    return in_maps


_NC_CACHE = {}


def kernel(x, rope_cos, rope_sin, qkv_w, qkv_b, out_w, out_b):
    B, T, D = x.shape
    H = 16
    NT = B * T
    S = NT // N_CORES
    key = (B, T, D, H)
    if key not in _NC_CACHE:
        _NC_CACHE[key] = build_nc(B, T, D, H)
    nc = _NC_CACHE[key]
    in_maps = _prep_inputs(
        np.asarray(x), np.asarray(rope_cos), np.asarray(rope_sin),
        np.asarray(qkv_w), np.asarray(qkv_b), np.asarray(out_w),
        np.asarray(out_b), B, T, D, H,
    )
    res = run_bass_kernel_spmd(nc, in_maps, core_ids=list(range(N_CORES)))
    out = np.empty((NT, D), np.float32)
    for c in range(N_CORES):
        out[c * S:(c + 1) * S] = res.results[c]["out"]
    return out.reshape(B, T, D)



# revision 30
# speedup vs baseline: 1.0384x; 1.0384x over previous
"""Causal self-attention with RoPE on 8 Trainium2 NeuronCores.

Sharding: tensor-parallel over heads (2 heads/core) through QKV projection,
RoPE and attention; AllToAll reshards attention output from head-split to
token-split; out-projection is token-parallel with full out_w per core
(no reduction needed). Output: each core produces its 512-token slice.

Schedule: QKV-projection blocks and attention chunks are fused into one
software-pipelined stream (see the comment at the fused loop); the softmax
denominator is accumulated on DVE (bf16 4x) and finished on Pool
(partition_all_reduce) so the PE only runs projections, scores, PV and the
out-projection.

Layouts (per core, f = feature, t = token, d = contraction):
  xT   [D, NT]    input transposed (d on partitions) - rhs/lhsT for projections
  q/k  [128, NT]  per-head, head-dim on partitions ("qT"): proj out [f, t]
  v    [NT, 256]  token-major: proj out [t, f]
  sT   [j, r]     scores transposed: lhsT=kT-tile, rhs=qT-block
  pT   [j, r]     exp(scores*scale) bf16, causally trimmed to r >= 128*(jt-4rb)
  oT   [dv, r]    PV: lhsT=v-tile [j, dv], rhs=pT [j, r]
  dacc [j, r]     bf16 elementwise-accumulated pT (denominator before reduce)
  out  [t, e]     out-proj: lhsT=attnT-tile [dv, t], rhs=out_wT [dv, e]
"""
import math
import numpy as np
import ml_dtypes

import concourse.bass as bass
import concourse.mybir as mybir
import concourse.tile as tile
from concourse import bacc, bass_isa
from concourse.bass_utils import run_bass_kernel_spmd

F32 = mybir.dt.float32
F32R = mybir.dt.float32r
BF16 = mybir.dt.bfloat16
AF = mybir.ActivationFunctionType
ALU = mybir.AluOpType

N_CORES = 8


def legalize_waits(nc, max_waits=1):
    """This walrus build only encodes one sync-wait per TPB instruction.
    Move extra waits emitted by Tile onto same-engine NoOps inserted
    immediately before the instruction."""
    n_split = 0
    for fn in nc.m.functions:
        for bb in fn.blocks:
            new_insts = []
            for inst in bb.instructions:
                si = getattr(inst, "sync_info", None)
                waits = list(si.on_wait) if si is not None and si.on_wait else []
                if len(waits) > max_waits and type(inst).__name__ != "InstNoOp":
                    extra, keep = waits[:-max_waits], waits[-max_waits:]
                    for k, w in enumerate(extra):
                        nop = mybir.InstNoOp(
                            name=f"{inst.name}_waitnop{k}",
                            engine=inst.engine,
                            ins=[],
                            outs=[],
                            sync_info=mybir.SyncInfo(on_wait=[w], on_update=[]),
                        )
                        nc.register_instruction(nop)
                        new_insts.append(nop)
                    inst.sync_info = mybir.SyncInfo(
                        on_wait=keep, on_update=list(si.on_update)
                    )
                    n_split += 1
                new_insts.append(inst)
            bb.instructions = new_insts
    return n_split


def build_nc(B=2, T=2048, D=2048, H=16, fake_cc=False, n_loop=1, dummy_io=False):
    HD = D // H                  # 128, head dim
    NT = B * T                   # total tokens
    HPC = H // N_CORES           # heads per core (2)
    DC = HPC * HD                # head channels per core (256)
    KT = D // 128                # contraction tiles for projections (16)
    NB = NT // 512               # 512-token blocks overall (8)
    RB = T // 512                # 512-token blocks per batch element (4)
    S = NT // N_CORES            # AllToAll shard = tokens per core (512)
    EB = D // 512                # 512-wide out-feature blocks (4)
    SCALE = 1.0 / math.sqrt(HD)

    nc = bacc.Bacc("TRN2", target_bir_lowering=False, debug=False, num_devices=N_CORES)
    # dummy_io: declare data tensors as internal DRAM (uninitialized) so the
    # timing NEFF has no big inputs to ship through the axon tunnel.
    ik = {"kind": "ExternalInput"} if not dummy_io else {}
    xT_e = nc.dram_tensor("xT", [D, NT], BF16, **ik)
    wqk_e = nc.dram_tensor("wqk", [D, 4 * HD], BF16, **ik)
    bqk_e = nc.dram_tensor("bqk", [4 * HD], F32, **ik)
    wv_e = nc.dram_tensor("wv", [D, DC], BF16, **ik)
    bv_e = nc.dram_tensor("bv", [DC], F32, **ik)
    cos_e = nc.dram_tensor("cosT", [HD, NT], BF16, **ik)
    sin_e = nc.dram_tensor("sinT", [HD, NT], BF16, **ik)
    masks_e = nc.dram_tensor("masks", [4, 128, 512], BF16, **ik)
    owT_e = nc.dram_tensor("owT", [D, D], BF16, **ik)
    ob_e = nc.dram_tensor("ob", [D], F32, **ik)
    out_e = nc.dram_tensor("out", [S, D], F32, kind="ExternalOutput")

    with tile.TileContext(nc) as tc:
      for _it in range(n_loop):
        with tc.tile_pool(name=f"persist{_it}", bufs=1) as pp, \
             tc.tile_pool(name=f"ow0{_it}", bufs=1) as owp0, \
             tc.tile_pool(name=f"dram{_it}", bufs=1, space="DRAM") as dp:
            # ---- persistent tiles ----
            qk = [pp.tile([128, NT], BF16, tag=f"qk{m}", name=f"qk{m}") for m in range(4)]
            v_sb = pp.tile([128, NT // 128, DC], BF16, tag="v", name="v")
            masks = pp.tile([128, 4, 512], BF16, tag="masks", name="masks")
            bqk = pp.tile([128, 4], F32, tag="bqk", name="bqk")
            nc.sync.dma_start(bqk[:], bqk_e.rearrange("(m p) -> p m", p=128))
            bv1 = pp.tile([1, DC], F32, tag="bv1", name="bv1")
            nc.sync.dma_start(bv1[:], bv_e[None, :])
            bvB = pp.tile([128, DC], F32, tag="bvB", name="bvB")
            nc.gpsimd.partition_broadcast(bvB[:], bv1[:])
            ob1 = pp.tile([1, D], F32, tag="ob1", name="ob1")
            obB = pp.tile([128, D], F32, tag="obB", name="obB")

            Zs = [dp.tile([N_CORES, HD, S], BF16, tag=f"Z{i}", name=f"Z{i}")
                  for i in range(HPC)]
            ZGs = [dp.tile([N_CORES, HD, S], BF16, tag=f"ZG{i}", name=f"ZG{i}")
                   for i in range(HPC)]

            zg_sb = [pp.tile([128, N_CORES, S], BF16, tag=f"zg{i}", name=f"zg{i}")
                     for i in range(HPC)]

            # ---- fused phases 1+2: QKV projection blocks interleaved with
            # attention chunks ----
            # Phase 1 alone is ~100% PE-bound while ACT/DVE idle; phase 2
            # alone is ACT/DVE-chain-bound with the PE only ~55% busy. After
            # projection block i (tokens [512i, 512i+512)), the attention
            # row-block (b=i//4, rb=i%4) has all its keys/values, so its
            # chunks are interleaved between block i+1's projection units:
            # the exp/mask/denominator chain hides under projection matmuls.
            # Attention work is emitted per single key-tile (128 keys x 512
            # queries) with a K_SKEW-deep software pipeline so the in-order
            # PE never waits on the ACT exp. Causal trim: key-tile jt only
            # serves queries q >= 128*(jt-4rb). The softmax denominator is
            # accumulated on DVE (bf16 4x mode) and finished on Pool
            # (partition_all_reduce + reciprocal) - no PE involvement.
            K_SKEW = 3
            with tc.tile_pool(name=f"p1w{_it}", bufs=1) as wp, \
                 tc.tile_pool(name=f"p1x{_it}", bufs=2) as xp, \
                 tc.tile_pool(name=f"p1t{_it}", bufs=3) as tp, \
                 tc.tile_pool(name=f"p2t{_it}", bufs=4) as tp2, \
                 tc.tile_pool(name=f"p2r{_it}", bufs=2) as tpr, \
                 tc.tile_pool(name=f"p1ps{_it}", bufs=2, space="PSUM") as ps1, \
                 tc.tile_pool(name=f"p2ps{_it}", bufs=4, space="PSUM") as ps2, \
                 tc.tile_pool(name=f"p2po{_it}", bufs=2, space="PSUM") as ps2o:
                wqk_sb = wp.tile([128, KT, 4 * HD], BF16, tag="wqk", name="wqk")
                wqk_r = wqk_e.rearrange("(kt p) f -> p kt f", p=128)
                for m2 in range(2):
                    nc.scalar.dma_start(
                        wqk_sb[:, :, bass.ts(m2, 256)], wqk_r[:, :, bass.ts(m2, 256)])
                # cos/sin halves: batch-0 tokens are needed ~25us in
                # (RoPE of block 0 gates the first attention chunks), batch-1
                # tokens only after ~80us. wv by the first v unit (~14us).
                cos_sb = wp.tile([128, NT], BF16, tag="cos", name="cos")
                sin_sb = wp.tile([128, NT], BF16, tag="sin", name="sin")
                wv_sb = wp.tile([128, KT, DC], BF16, tag="wv", name="wv")
                nc.scalar.dma_start(cos_sb[:, :T], cos_e[:, :T])
                nc.scalar.dma_start(sin_sb[:, :T], sin_e[:, :T])
                nc.scalar.dma_start(wv_sb[:], wv_e.rearrange("(kt p) f -> p kt f", p=128))
                nc.scalar.dma_start(cos_sb[:, T:], cos_e[:, T:])
                nc.scalar.dma_start(sin_sb[:, T:], sin_e[:, T:])

                state = {}

                def emit_p1_unit(blk, u, xb):
                    tsl = bass.ts(blk, 512)
                    psu = ps1.tile([128, 512], F32, tag="ps", name="ps")
                    if u < 4:
                        m = u
                        for kt in range(KT):
                            nc.tensor.matmul(
                                psu[:],
                                wqk_sb[:, kt, bass.ts(m, 128)],
                                xb[:, kt, :],
                                start=(kt == 0),
                                stop=(kt == KT - 1),
                            )
                        nc.scalar.activation(
                            qk[m][:, tsl], psu[:], AF.Identity,
                            bias=bqk[:, m:m + 1], scale=1.0,
                        )
                        # RoPE in place: qk = qk*cos + swap(qk)*s2, where
                        # s2 = sin with first half negated (host-prepared) and
                        # swap exchanges partition halves (engines cannot read
                        # across partitions -> use SBUF->SBUF DMA).
                        qm = qk[m][:, tsl]
                        qsw = tp.tile([128, 512], BF16, tag="qsw", name="qsw")
                        nc.sync.dma_start(qsw[0:64, :], qm[64:128, :])
                        nc.sync.dma_start(qsw[64:128, :], qm[0:64, :])
                        nc.vector.tensor_mul(qsw[:], qsw[:], sin_sb[:, tsl])
                        nc.vector.tensor_mul(qm, qm, cos_sb[:, tsl])
                        nc.vector.tensor_add(qm, qm, qsw[:])
                    else:
                        tt = u - 4
                        for kt in range(KT):
                            nc.tensor.matmul(
                                psu[:, :DC],
                                xb[:, kt, bass.ts(tt, 128)],
                                wv_sb[:, kt, :],
                                start=(kt == 0),
                                stop=(kt == KT - 1),
                            )
                        nc.vector.tensor_add(v_sb[:, blk * 4 + tt, :], psu[:, :DC], bvB[:])

                def a_chunks(blk):
                    b, rb = blk // 4, blk % 4
                    return [(hh, b, rb, jt) for hh in range(HPC)
                            for jt in range(4 * rb + 4)]

                def emit_ascores(c):
                    hh, b, rb, jt = c
                    q0 = max(jt - 4 * rb, 0) * 128
                    pss = ps2.tile([128, 512], F32, tag="pss", name="pss")
                    nc.tensor.matmul(
                        pss[:, bass.ds(q0, 512 - q0)],
                        qk[2 + hh][:, bass.ds(b * T + jt * 128, 128)],
                        qk[hh][:, bass.ds(b * T + rb * 512 + q0, 512 - q0)],
                        start=True, stop=True,
                    )
                    state[c] = pss

                def emit_apost(c):
                    hh, b, rb, jt = c
                    pss = state.pop(c)
                    njt = 4 * rb + 4
                    if jt == 0:
                        state[c[:3]] = (
                            ps2o.tile([128, 512], F32, tag="pso", name="pso"),
                            tpr.tile([128, 512], BF16, tag="dacc", name="dacc"),
                        )
                    pso, dacc = state[c[:3]]
                    mh = jt - 4 * rb
                    q0 = max(mh, 0) * 128
                    csl = bass.ds(q0, 512 - q0)
                    pT = tp2.tile([128, 512], BF16, tag="pT", name="pT")
                    nc.scalar.activation(pT[:, csl], pss[:, csl], AF.Exp, scale=SCALE)
                    if mh >= 0:
                        nc.vector.tensor_mul(pT[:, csl], pT[:, csl], masks[:, mh, csl])
                    if jt == 0:
                        nc.vector.tensor_copy(dacc[:], pT[:])
                    else:
                        nc.vector.tensor_add(dacc[:, csl], dacc[:, csl], pT[:, csl])
                    nc.tensor.matmul(
                        pso[:, csl],
                        v_sb[:, (b * T) // 128 + jt, bass.ts(hh, HD)],
                        pT[:, csl],
                        start=(jt == 0),
                        stop=(jt == njt - 1),
                    )
                    if jt != njt - 1:
                        return
                    del state[c[:3]]
                    # denominator finish on Pool (off the PE path): all-reduce
                    # over partitions -> reciprocal -> scale
                    denB = tpr.tile([128, 512], F32, tag="denB", name="denB")
                    nc.gpsimd.partition_all_reduce(
                        denB[:], dacc[:], 128, bass_isa.ReduceOp.add)
                    recipB = tpr.tile([128, 512], F32, tag="recipB", name="recipB")
                    nc.vector.reciprocal(recipB[:], denB[:])
                    oT = tpr.tile([128, 512], BF16, tag="oT", name="oT")
                    nc.vector.tensor_mul(oT[:], pso[:], recipB[:])
                    g = b * RB + rb
                    for off in range(0, 512, S):
                        sh = (g * 512 + off) // S
                        nc.sync.dma_start(
                            Zs[hh][sh, :, bass.ds((g * 512 + off) % S, min(512, S))],
                            oT[:, bass.ds(off, min(512, S))],
                        )
                    if b == B - 1 and rb == RB - 1:
                        # reshard this head-half; phase 4 consumes it
                        if fake_cc:
                            nc.scalar.dma_start(ZGs[hh][:], Zs[hh][:])
                        else:
                            nc.gpsimd.collective_compute(
                                "AllToAll", ALU.bypass,
                                replica_groups=[list(range(N_CORES))],
                                ins=[Zs[hh][:]], outs=[ZGs[hh][:]],
                            )
                        nc.sync.dma_start(
                            zg_sb[hh][:], ZGs[hh][:].rearrange("c d s -> d c s")
                        )

                pend = []

                def push_chunk(c):
                    emit_ascores(c)
                    pend.append(c)
                    if len(pend) > K_SKEW:
                        emit_apost(pend.pop(0))

                last = a_chunks(NB - 1)
                # hh0 chunks of the last block whose KEY tile lives in an
                # earlier block interleave with the last block's v-projection
                # units; ALL hh1 chunks stay after hh0's AllToAll so the
                # collective flies under real attention work
                last_early = [c for c in last if c[3] < 4 * (RB - 1) and c[0] == 0]
                last_late = [c for c in last if c[3] >= 4 * (RB - 1) or c[0] != 0]
                n_le, le_done = len(last_early), 0
                for blk in range(NB):
                    xb = xp.tile([128, KT, 512], BF16, tag="xb", name="xb")
                    xTr = xT_e.rearrange("(kt p) t -> p kt t", p=128)
                    tsl = bass.ts(blk, 512)
                    npc = 4 if blk == 0 else 2
                    for pc in range(npc):
                        ksl = bass.ts(pc, KT // npc)
                        nc.sync.dma_start(xb[:, ksl, :], xTr[:, ksl, tsl])
                    if blk == 0:
                        nc.sync.dma_start(masks[:], masks_e.rearrange("m p c -> p m c"))
                    ach = a_chunks(blk - 1) if blk > 0 else []
                    done = 0
                    for u in range(8):
                        emit_p1_unit(blk, u, xb)
                        # at the last block, drain the previous block's chunks
                        # by unit 3 so its own early chunks (keys in earlier
                        # blocks) can follow without interleaving groups
                        den = 4 if blk == NB - 1 else 8
                        want = min(len(ach), (len(ach) * (u + 1) + den - 1) // den)
                        while done < want:
                            push_chunk(ach[done])
                            done += 1
                        if blk == NB - 1 and u >= 4:
                            want_e = (n_le * (u - 3) + 3) // 4
                            while le_done < want_e:
                                push_chunk(last_early[le_done])
                                le_done += 1
                nc.scalar.dma_start(ob1[:], ob_e[None, :])
                nc.gpsimd.partition_broadcast(obB[:], ob1[:])
                # prefetch the first out-proj weight tile (pass 1, e=0)
                # while the attention tail drains
                ow0 = owp0.tile([128, KT // 2, 512], BF16, tag="ow0", name="ow0")
                owr0 = owT_e.rearrange("(kt p) f -> p kt f", p=128)[:, :, bass.ts(0, 512)]
                nc.scalar.dma_start(ow0[:], owr0[:, 0::2, :])
                for c in last_early[le_done:] + last_late:
                    push_chunk(c)
                while pend:
                    emit_apost(pend.pop(0))

            # ---- phase 4: out projection on own token slice ----
            # Two-pass contraction: all zg0 (first AllToAll) partial sums are
            # computed and evicted before any zg1 tile is touched, so the
            # second AllToAll and the zg1 load hide behind real matmul work.
            with tc.tile_pool(name=f"p4z{_it}", bufs=1) as zp, \
                 tc.tile_pool(name=f"p4w{_it}", bufs=3) as owp, \
                 tc.tile_pool(name=f"p4t{_it}", bufs=4) as tp4, \
                 tc.tile_pool(name=f"p4ps{_it}", bufs=4, space="PSUM") as ps4:
                accbig = zp.tile([128, EB * (S // 128), 512], F32, tag="accbig", name="accbig")
                zgs = zg_sb
                for e in range(EB):
                    if e == 0:
                        ow = ow0
                    else:
                        ow = owp.tile([128, KT // 2, 512], BF16, tag="ow", name="ow")
                        owr = owT_e.rearrange("(kt p) f -> p kt f", p=128)[:, :, bass.ts(e, 512)]
                        ow_even = owr[:, 0::2, :]
                        nc.scalar.dma_start(ow[:, :KT // 4, :], ow_even[:, :KT // 4, :])
                        nc.scalar.dma_start(ow[:, KT // 4:, :], ow_even[:, KT // 4:, :])
                    for tt in range(S // 128):
                        pso4 = ps4.tile([128, 512], F32, tag="ps4", name="ps4")
                        for zt in range(KT // HPC):
                            nc.tensor.matmul(
                                pso4[:],
                                zgs[0][:, zt, bass.ts(tt, 128)],
                                ow[:, zt, :],
                                start=(zt == 0),
                                stop=(zt == KT // HPC - 1),
                            )
                        nc.scalar.activation(
                            accbig[:, e * (S // 128) + tt, :], pso4[:], AF.Copy)
                for e in range(EB):
                    ow2 = owp.tile([128, KT // 2, 512], BF16, tag="ow2", name="ow2")
                    owr = owT_e.rearrange("(kt p) f -> p kt f", p=128)[:, :, bass.ts(e, 512)]
                    ow_odd = owr[:, 1::2, :]
                    nc.scalar.dma_start(ow2[:, :KT // 4, :], ow_odd[:, :KT // 4, :])
                    nc.scalar.dma_start(ow2[:, KT // 4:, :], ow_odd[:, KT // 4:, :])
                    for tt in range(S // 128):
                        pso4 = ps4.tile([128, 512], F32, tag="ps4", name="ps4")
                        for zt in range(KT // HPC):
                            nc.tensor.matmul(
                                pso4[:],
                                zgs[1][:, zt, bass.ts(tt, 128)],
                                ow2[:, zt, :],
                                start=(zt == 0),
                                stop=(zt == KT // HPC - 1),
                            )
                        mid = tp4.tile([128, 512], F32, tag="mid", name="mid")
                        nc.vector.tensor_add(mid[:], pso4[:], accbig[:, e * (S // 128) + tt, :])
                        of = tp4.tile([128, 512], F32, tag="of", name="of")
                        nc.vector.tensor_add(of[:], mid[:], obB[:, bass.ts(e, 512)])
                        nc.sync.dma_start(out_e[bass.ts(tt, 128), bass.ts(e, 512)], of[:])

    nc.compile()          # Bacc pass pipeline (library loads, nop fusion, regs)
    legalize_waits(nc)    # must run after all nop-fusion passes
    bass.Bass.finalize(nc)  # freeze without re-running Bacc compile
    return nc


def _prep_inputs(x, rope_cos, rope_sin, qkv_w, qkv_b, out_w, out_b, B, T, D, H):
    HD = D // H
    NT = B * T
    HPC = H // N_CORES
    bf = ml_dtypes.bfloat16

    x2 = np.ascontiguousarray(x.reshape(NT, D).T).astype(bf)           # [D, NT]
    cosT = np.ascontiguousarray(
        np.tile(rope_cos[0, 0].T, (1, B))).astype(bf)                   # [HD, NT]
    s2 = np.tile(rope_sin[0, 0].T, (1, B)).copy()
    s2[:HD // 2] *= -1.0
    sinT = np.ascontiguousarray(s2).astype(bf)
    owT = np.ascontiguousarray(out_w.T).astype(bf)                      # [D, D]
    ob = out_b.astype(np.float32)

    c_grid = np.arange(512)[None, :]
    p_grid = np.arange(128)[:, None]
    masks = np.stack(
        [(c_grid >= 128 * m + p_grid) for m in range(4)]
    ).astype(bf)                                                        # [4,128,512]

    in_maps = []
    for c in range(N_CORES):
        heads = [HPC * c + i for i in range(HPC)]
        q_rows = np.concatenate([qkv_w[h * HD:(h + 1) * HD] for h in heads])
        k_rows = np.concatenate([qkv_w[D + h * HD:D + (h + 1) * HD] for h in heads])
        v_rows = np.concatenate([qkv_w[2 * D + h * HD:2 * D + (h + 1) * HD] for h in heads])
        wqk = np.ascontiguousarray(np.concatenate([q_rows, k_rows]).T).astype(bf)
        wv = np.ascontiguousarray(v_rows.T).astype(bf)
        bq = np.concatenate([qkv_b[h * HD:(h + 1) * HD] for h in heads])
        bk = np.concatenate([qkv_b[D + h * HD:D + (h + 1) * HD] for h in heads])
        bqk = np.concatenate([bq, bk]).astype(np.float32)
        bv = np.concatenate(
            [qkv_b[2 * D + h * HD:2 * D + (h + 1) * HD] for h in heads]
        ).astype(np.float32)
        in_maps.append({
            "xT": x2, "wqk": wqk, "bqk": bqk, "wv": wv, "bv": bv,
            "cosT": cosT, "sinT": sinT, "masks": masks,
            "owT": owT, "ob": ob,
        })
    return in_maps


_NC_CACHE = {}


def kernel(x, rope_cos, rope_sin, qkv_w, qkv_b, out_w, out_b):
    B, T, D = x.shape
    H = 16
    NT = B * T
    S = NT // N_CORES
    key = (B, T, D, H)
    if key not in _NC_CACHE:
        _NC_CACHE[key] = build_nc(B, T, D, H)
    nc = _NC_CACHE[key]
    in_maps = _prep_inputs(
        np.asarray(x), np.asarray(rope_cos), np.asarray(rope_sin),
        np.asarray(qkv_w), np.asarray(qkv_b), np.asarray(out_w),
        np.asarray(out_b), B, T, D, H,
    )
    res = run_bass_kernel_spmd(nc, in_maps, core_ids=list(range(N_CORES)))
    out = np.empty((NT, D), np.float32)
    for c in range(N_CORES):
        out[c * S:(c + 1) * S] = res.results[c]["out"]
    return out.reshape(B, T, D)

